# revision 1
# baseline (speedup 1.0000x reference)
"""Trainium2 Bass kernel for nn_ComboLoss (MTP loss + BCE loss).

Data-parallel over 8 NeuronCores: each core processes 8192 rows of the
65536-row batch and produces two partial sums [sum(ce + reg), sum(bce_raw)];
the host combines them into the final scalar loss.

Per-core layout: 8 supertiles of 1024 rows; each supertile maps G=8
consecutive rows onto each of the 128 SBUF partitions.  The per-supertile
loop does only the big dense work (deltas, squares, sqrt, per-mode distance
sums); everything per-row-small (eligibility, argmin, cross-entropy pieces)
runs once per core over all 64 row-groups, and the best-mode trajectory is
fetched with one indirect DMA (8192 row-gathers) fused with the "- gt"
subtract via the DMA compute-add against a host-negated gt.

NOTE: the "path_gt" DRAM input holds the NEGATED ground truth; the host
wrapper negates it.  All uses inside account for the sign flip.
"""

import math
import os
import sys
from contextlib import ExitStack

import numpy as np

for _p in ("/opt/trn_rl_repo", "/root/.axon_site/_ro/trn_rl_repo"):
    if os.path.isdir(_p) and _p not in sys.path:
        sys.path.insert(0, _p)
        break

import concourse.bass as bass
import concourse.bacc as bacc
import concourse.mybir as mybir
import concourse.tile as tile
from concourse.bass_utils import run_bass_kernel_spmd

F32 = mybir.dt.float32
I32 = mybir.dt.int32
ALU = mybir.AluOpType
ACTF = mybir.ActivationFunctionType
AX = mybir.AxisListType

B = 65536
NCORES = 8
BLOC = B // NCORES          # 8192 rows per core
P = 128                     # SBUF partitions
G = 8                       # row-groups per partition per supertile
ROWS_SUP = P * G            # 1024 rows per supertile
NSUP = BLOC // ROWS_SUP     # 8 supertiles
NM = 5                      # modes
T = 50                      # waypoints
T2 = 2 * T                  # 100 coords per trajectory
F = NM * T2 + NM            # 505 features in path_pred
NJ = NSUP * G               # 64 row-groups per partition over the whole core

BIG = 1.0e30
INV_COS5SQ = float(1.0 / (math.cos(math.radians(5.0)) ** 2))


def _build_bass():
    nc = bacc.Bacc("TRN2", target_bir_lowering=False, debug=False)

    pred_d = nc.dram_tensor("path_pred", [BLOC, F], F32, kind="ExternalInput").ap()
    gt_d = nc.dram_tensor("path_gt", [BLOC, T2], F32, kind="ExternalInput").ap()
    crp_d = nc.dram_tensor("cr_pred", [P, NJ], F32, kind="ExternalInput").ap()
    crg_d = nc.dram_tensor("cr_gt", [P, NJ], F32, kind="ExternalInput").ap()
    rnd_d = nc.dram_tensor("rand_modes", [P, NJ], F32, kind="ExternalInput").ap()
    out_d = nc.dram_tensor("partials", [1, 2], F32, kind="ExternalOutput").ap()

    with tile.TileContext(nc) as tc, ExitStack() as ctx:
        cpool = ctx.enter_context(tc.tile_pool(name="const", bufs=1))
        inp = ctx.enter_context(tc.tile_pool(name="inp", bufs=2))
        wrk = ctx.enter_context(tc.tile_pool(name="wrk", bufs=2))
        sml = ctx.enter_context(tc.tile_pool(name="sml", bufs=1))
        pps = ctx.enter_context(tc.tile_pool(name="pps", bufs=1, space="PSUM"))

        # ---- constants ----
        iota_i = cpool.tile([P, NM], I32)
        nc.gpsimd.iota(iota_i[:], pattern=[[1, NM]], base=0, channel_multiplier=0)
        iota_a = cpool.tile([P, NM], F32)          # [0,1,2,3,4]
        nc.vector.tensor_copy(iota_a[:], iota_i[:])
        iota_di = cpool.tile([P, NM], I32)
        nc.gpsimd.iota(iota_di[:], pattern=[[-1, NM]], base=NM, channel_multiplier=0)
        iota_d = cpool.tile([P, NM], F32)          # [5,4,3,2,1]
        nc.vector.tensor_copy(iota_d[:], iota_di[:])
        ones = cpool.tile([P, 1], F32)
        nc.vector.memset(ones[:], 1.0)
        negone = cpool.tile([P, 1], F32)
        nc.vector.memset(negone[:], -1.0)
        # element offset of each row-group's trajectory block: row*505
        # (row = i*1024 + p*8 + g for j = i*8+g)
        rb_i = cpool.tile([P, NJ], I32)
        nc.gpsimd.iota(
            rb_i[:],
            pattern=[[ROWS_SUP, NSUP], [1, G]],
            base=0,
            channel_multiplier=G,
        )
        rb_f = cpool.tile([P, NJ], F32)
        nc.vector.tensor_copy(rb_f[:], rb_i[:])
        nc.vector.tensor_scalar(rb_f[:], rb_f[:], float(F), None, ALU.mult)

        # ---- resident inputs ----
        rnd_sb = cpool.tile([P, NJ], F32)
        nc.sync.dma_start(rnd_sb[:], rnd_d)
        crp_sb = cpool.tile([P, NJ], F32)
        nc.sync.dma_start(crp_sb[:], crp_d)
        crg_sb = cpool.tile([P, NJ], F32)
        nc.sync.dma_start(crg_sb[:], crg_d)
        # whole negated-gt shard, laid out (i, g, t2) per partition
        gtB = cpool.tile([P, NJ * T2], F32)
        gt_src = gt_d.rearrange("(i p g) t -> p i g t", i=NSUP, p=P, g=G)
        nc.sync.dma_start(gtB[:], gt_src)
        gtJ = gtB[:].rearrange("p (j t) -> p j t", j=NJ)       # (P, NJ, T2)

        # ---- residents produced by the loop ----
        distB = cpool.tile([P, NJ * NM], F32)
        sqlB = cpool.tile([P, NJ * NM * 2], F32)
        tlB = cpool.tile([P, NJ * NM * 2], F32)
        lgB = cpool.tile([P, NJ * NM], F32)
        shB = cpool.tile([P, NJ * NM], F32)
        stack2 = cpool.tile([P, 2], F32)

        # ============ Phase A: per-supertile dense work ============
        for i in range(NSUP):
            rsl = slice(i * ROWS_SUP, (i + 1) * ROWS_SUP)

            pred_t = inp.tile([P, G * F], F32, tag="pred")
            nc.sync.dma_start(
                pred_t[:], pred_d[rsl, :].rearrange("(p g) f -> p (g f)", p=P)
            )
            predg = pred_t[:].rearrange("p (g f) -> p g f", g=G)
            traj4 = predg[:, :, 0:NM * T2].rearrange("p g (m t) -> p g m t", m=NM)
            logits = predg[:, :, NM * T2:F]                     # (P, G, NM)
            gt3 = gtB[:, i * G * T2:(i + 1) * G * T2].rearrange(
                "p (g t) -> p g t", g=G
            )                                                   # negated gt

            # deltas d = traj + (-gt)  (gpsimd, one broadcast op)
            d_t = wrk.tile([P, G * NM * T2], F32, tag="d")
            d4 = d_t[:].rearrange("p (g m t) -> p g m t", g=G, m=NM)
            gt_b = gt3.unsqueeze(2).broadcast_to((P, G, NM, T2))
            nc.gpsimd.tensor_add(d4, traj4, gt_b)

            # squares (in place), per-waypoint distance^2, sqrt, sum over t
            nc.scalar.activation(d_t[:], d_t[:], ACTF.Square)
            s4 = d_t[:].rearrange("p (gm t c) -> p gm t c", gm=G * NM, t=T, c=2)
            e_t = wrk.tile([P, G * NM * T], F32, tag="e")
            e3 = e_t[:].rearrange("p (gm t) -> p gm t", gm=G * NM)
            nc.vector.tensor_add(e3, s4[:, :, :, 0], s4[:, :, :, 1])
            nc.scalar.activation(e_t[:], e_t[:], ACTF.Sqrt)
            nc.vector.tensor_reduce(
                distB[:, i * G * NM:(i + 1) * G * NM], e3, axis=AX.X, op=ALU.add
            )

            # stash last-waypoint data + logits for the batched small phase
            tl2 = traj4[:, :, :, T2 - 2:T2]                     # (P,G,NM,2)
            sql_s = sqlB[:, i * G * NM * 2:(i + 1) * G * NM * 2].rearrange(
                "p (g m c) -> p g m c", g=G, m=NM
            )
            nc.scalar.activation(sql_s, tl2, ACTF.Square)
            tl_s = tlB[:, i * G * NM * 2:(i + 1) * G * NM * 2].rearrange(
                "p (g m c) -> p g m c", g=G, m=NM
            )
            nc.gpsimd.tensor_copy(tl_s, tl2)
            nc.gpsimd.tensor_copy(
                lgB[:, i * G * NM:(i + 1) * G * NM].rearrange(
                    "p (g m) -> p g m", g=G
                ),
                logits,
            )

        # ============ Phase B: batched per-row small ops ============
        sqlJ = sqlB[:].rearrange("p (j m c) -> p j m c", j=NJ, m=NM)
        tlJ = tlB[:].rearrange("p (j m c) -> p j m c", j=NJ, m=NM)
        lgJ = lgB[:].rearrange("p (j m) -> p j m", j=NJ)
        distJ = distB[:].rearrange("p (j m) -> p j m", j=NJ)

        nt2 = sml.tile([P, NJ * NM], F32)
        nt2J = nt2[:].rearrange("p (j m) -> p j m", j=NJ)
        nc.vector.tensor_add(nt2J, sqlJ[:, :, :, 0], sqlJ[:, :, :, 1])
        gl2 = gtJ[:, :, T2 - 2:T2]                              # (P,NJ,2) negated
        gg = sml.tile([P, NJ * 2], F32)
        ggJ = gg[:].rearrange("p (j c) -> p j c", j=NJ)
        nc.vector.tensor_mul(ggJ, gl2, gl2)
        nr2 = sml.tile([P, NJ], F32)
        nc.vector.tensor_add(nr2[:], ggJ[:, :, 0], ggJ[:, :, 1])

        tx = tlJ[:, :, :, 0]
        ty = tlJ[:, :, :, 1]
        rx_b = gtJ[:, :, T2 - 2:T2 - 1].broadcast_to((P, NJ, NM))
        ry_b = gtJ[:, :, T2 - 1:T2].broadcast_to((P, NJ, NM))
        a1 = sml.tile([P, NJ * NM], F32)
        a1J = a1[:].rearrange("p (j m) -> p j m", j=NJ)
        nc.vector.tensor_mul(a1J, tx, rx_b)
        a2 = sml.tile([P, NJ * NM], F32)
        a2J = a2[:].rearrange("p (j m) -> p j m", j=NJ)
        nc.vector.tensor_mul(a2J, ty, ry_b)
        dot = sml.tile([P, NJ * NM], F32)                       # = -(true dot)
        nc.vector.tensor_add(dot[:], a1[:], a2[:])

        rhs = sml.tile([P, NJ * NM], F32)
        rhsJ = rhs[:].rearrange("p (j m) -> p j m", j=NJ)
        nr2_b = nr2[:].unsqueeze(2).broadcast_to((P, NJ, NM))
        nc.vector.tensor_mul(rhsJ, nt2J, nr2_b)
        dot2c = sml.tile([P, NJ * NM], F32)
        nc.vector.scalar_tensor_tensor(
            dot2c[:], dot[:], INV_COS5SQ, dot[:], ALU.mult, ALU.mult
        )
        e1 = sml.tile([P, NJ * NM], F32)
        nc.vector.tensor_tensor(e1[:], dot2c[:], rhs[:], ALU.is_ge)
        elig = sml.tile([P, NJ * NM], F32)
        # true dot > 0  <=>  negated dot < 0
        nc.vector.scalar_tensor_tensor(
            elig[:], dot[:], 0.0, e1[:], ALU.is_lt, ALU.mult
        )

        welig = sml.tile([P, NJ * NM], F32)
        nc.vector.tensor_scalar(welig[:], elig[:], -BIG, BIG, ALU.mult, ALU.add)
        score = sml.tile([P, NJ * NM], F32)
        scoreJ = score[:].rearrange("p (j m) -> p j m", j=NJ)
        nc.vector.tensor_add(score[:], distB[:], welig[:])
        minv = sml.tile([P, NJ], F32)
        nc.vector.tensor_reduce(minv[:], scoreJ, axis=AX.X, op=ALU.min)
        eq = sml.tile([P, NJ * NM], F32)
        eqJ = eq[:].rearrange("p (j m) -> p j m", j=NJ)
        minv_b = minv[:].unsqueeze(2).broadcast_to((P, NJ, NM))
        nc.vector.tensor_tensor(eqJ, scoreJ, minv_b, ALU.is_equal)
        wq = sml.tile([P, NJ * NM], F32)
        wqJ = wq[:].rearrange("p (j m) -> p j m", j=NJ)
        iotaD_b = iota_d[:].unsqueeze(1).broadcast_to((P, NJ, NM))
        nc.vector.tensor_tensor(wqJ, eqJ, iotaD_b, ALU.mult)
        mxw = sml.tile([P, NJ], F32)
        nc.vector.tensor_reduce(mxw[:], wqJ, axis=AX.X, op=ALU.max)
        bidx = sml.tile([P, NJ], F32)
        nc.vector.tensor_scalar(bidx[:], mxw[:], -1.0, float(NM), ALU.mult, ALU.add)
        anye = sml.tile([P, NJ], I32)
        nc.vector.tensor_scalar(anye[:], minv[:], BIG, None, ALU.is_lt)
        bf = sml.tile([P, NJ], F32)
        nc.vector.tensor_copy(bf[:], rnd_sb[:])
        nc.vector.copy_predicated(bf[:], anye[:], bidx[:])

        mask = sml.tile([P, NJ * NM], I32)
        maskJ = mask[:].rearrange("p (j m) -> p j m", j=NJ)
        iotaA_b = iota_a[:].unsqueeze(1).broadcast_to((P, NJ, NM))
        bf_b = bf[:].unsqueeze(2).broadcast_to((P, NJ, NM))
        nc.vector.tensor_tensor(maskJ, iotaA_b, bf_b, ALU.is_equal)

        # cross-entropy pieces (exp/ln deferred)
        mxl = sml.tile([P, NJ], F32)
        nc.vector.tensor_reduce(mxl[:], lgJ, axis=AX.X, op=ALU.max)
        shJ = shB[:].rearrange("p (j m) -> p j m", j=NJ)
        mxl_b = mxl[:].unsqueeze(2).broadcast_to((P, NJ, NM))
        nc.vector.tensor_sub(shJ, lgJ, mxl_b)
        lbt = sml.tile([P, NJ * NM], F32)
        lbtJ = lbt[:].rearrange("p (j m) -> p j m", j=NJ)
        nc.vector.tensor_mul(lbtJ, lgJ, maskJ)
        lb = sml.tile([P, NJ], F32)
        nc.vector.tensor_reduce(lb[:], lbtJ, axis=AX.X, op=ALU.add)
        mb = sml.tile([P, NJ], F32)
        nc.vector.tensor_sub(mb[:], mxl[:], lb[:])

        # ===== gather best trajectory: indirect DMA + fused "-gt" =====
        idxf = sml.tile([P, NJ], F32)
        nc.vector.scalar_tensor_tensor(
            idxf[:], bf[:], float(T2), rb_f[:], ALU.mult, ALU.add
        )
        idxi = sml.tile([P, NJ], I32)
        nc.vector.tensor_copy(idxi[:], idxf[:])

        db_t = cpool.tile([P, NJ * T2], F32)
        pred_flat = pred_d.rearrange("r f -> (r f)").unsqueeze(0)
        nc.gpsimd.indirect_dma_start(
            out=db_t[:],
            out_offset=None,
            in_=pred_flat,
            in_offset=bass.IndirectOffsetOnAxis(ap=idxi[:], axis=1),
        )
        nc.vector.tensor_add(db_t[:], db_t[:], gtB[:])          # d = traj + (-gt)

        # smooth-L1: sum(relu(|d|-1)) + 0.5*sum(min(|d|,1)^2), means folded in
        nc.scalar.activation(db_t[:], db_t[:], ACTF.Abs)        # ad in place
        t_t = cpool.tile([P, NJ * T2], F32)
        nc.scalar.activation(t_t[:], db_t[:], ACTF.Relu, bias=negone[:])
        tred = sml.tile([P, NJ], F32)
        nc.vector.tensor_reduce(
            tred[:], t_t[:].rearrange("p (j t) -> p j t", j=NJ), axis=AX.X,
            op=ALU.add,
        )
        nc.vector.tensor_scalar(t_t[:], db_t[:], 1.0, None, ALU.min)
        nc.scalar.activation(t_t[:], t_t[:], ACTF.Square)
        qred = sml.tile([P, NJ], F32)
        nc.vector.tensor_reduce(
            qred[:], t_t[:].rearrange("p (j t) -> p j t", j=NJ), axis=AX.X,
            op=ALU.add,
        )
        reg = sml.tile([P, NJ], F32)
        nc.vector.tensor_scalar(reg[:], tred[:], 1.0 / T2, None, ALU.mult)
        nc.vector.scalar_tensor_tensor(
            reg[:], qred[:], 0.5 / T2, reg[:], ALU.mult, ALU.add
        )

        # ============ Phase C: exp/ln + BCE + final reduce ============
        ex = sml.tile([P, NJ * NM], F32)
        nc.scalar.activation(ex[:], shB[:], ACTF.Exp)
        se = sml.tile([P, NJ], F32)
        nc.vector.tensor_reduce(
            se[:], ex[:].rearrange("p (j m) -> p j m", j=NJ), axis=AX.X, op=ALU.add
        )
        nc.scalar.activation(se[:], se[:], ACTF.Ln)             # lse (minus mx)
        ce = sml.tile([P, NJ], F32)
        nc.vector.tensor_add(ce[:], mb[:], se[:])
        nc.vector.tensor_add(ce[:], ce[:], reg[:])
        nc.vector.tensor_reduce(stack2[:, 0:1], ce[:], axis=AX.X, op=ALU.add)

        lp = sml.tile([P, NJ], F32)
        nc.scalar.activation(lp[:], crp_sb[:], ACTF.Ln)
        nc.vector.tensor_scalar(lp[:], lp[:], -100.0, None, ALU.max)
        om = sml.tile([P, NJ], F32)
        nc.vector.tensor_scalar(om[:], crp_sb[:], -1.0, 1.0, ALU.mult, ALU.add)
        nc.scalar.activation(om[:], om[:], ACTF.Ln)
        nc.vector.tensor_scalar(om[:], om[:], -100.0, None, ALU.max)
        u_t = sml.tile([P, NJ], F32)
        nc.vector.tensor_sub(u_t[:], lp[:], om[:])
        nc.vector.tensor_mul(u_t[:], crg_sb[:], u_t[:])
        nc.vector.tensor_add(u_t[:], u_t[:], om[:])
        nc.vector.tensor_reduce(stack2[:, 1:2], u_t[:], axis=AX.X, op=ALU.add)

        ps = pps.tile([1, 2], F32)
        nc.tensor.matmul(ps[:], ones[:], stack2[:], start=True, stop=True)
        fin = cpool.tile([1, 2], F32)
        nc.scalar.copy(fin[:], ps[:])
        nc.sync.dma_start(out_d, fin[:])

    nc.compile()
    return nc


_NC_CACHE = None


def _get_nc():
    global _NC_CACHE
    if _NC_CACHE is None:
        _NC_CACHE = _build_bass()
    return _NC_CACHE


def _rand_modes_full() -> np.ndarray:
    """The reference's fallback modes: jax.random.randint(key(42), (B,), 0, 5)."""
    import jax

    cpu = jax.devices("cpu")[0]
    with jax.default_device(cpu):
        r = jax.random.randint(jax.random.key(42), (B,), 0, NM)
        return np.asarray(jax.device_get(r)).astype(np.float32)


def _make_in_maps(path_pred, path_gt, cr_pred, cr_gt):
    pp = np.ascontiguousarray(np.asarray(path_pred, dtype=np.float32))
    # NOTE: negated — the kernel consumes -gt everywhere
    pg = np.ascontiguousarray(
        -np.asarray(path_gt, dtype=np.float32).reshape(B, T2)
    )
    crp = np.asarray(cr_pred, dtype=np.float32).reshape(B)
    crg = np.asarray(cr_gt, dtype=np.float32).reshape(B)
    rnd = _rand_modes_full()

    in_maps = []
    for c in range(NCORES):
        sl = slice(c * BLOC, (c + 1) * BLOC)
        rc = (
            rnd[sl]
            .reshape(NSUP, P, G)
            .transpose(1, 0, 2)
            .reshape(P, NJ)
        )
        in_maps.append(
            {
                "path_pred": pp[sl],
                "path_gt": pg[sl],
                "cr_pred": np.ascontiguousarray(crp[sl].reshape(P, NJ)),
                "cr_gt": np.ascontiguousarray(crg[sl].reshape(P, NJ)),
                "rand_modes": np.ascontiguousarray(rc),
            }
        )
    return in_maps


def _combine(results) -> np.float32:
    tot_main = 0.0
    tot_bce = 0.0
    for r in results:
        p = np.asarray(r["partials"], dtype=np.float64)
        tot_main += p[0, 0]
        tot_bce += p[0, 1]
    return np.float32(tot_main / B - tot_bce / B)


def kernel(path_pred, path_gt, cr_pred, cr_gt, log_vars=None, **_ignored):
    in_maps = _make_in_maps(path_pred, path_gt, cr_pred, cr_gt)
    nc = _get_nc()
    res = run_bass_kernel_spmd(nc, in_maps, list(range(NCORES)))
    return _combine(res.results)


def kernel_traced(path_pred, path_gt, cr_pred, cr_gt, log_vars=None, **kw):
    """Like kernel() but with NTFF profiling; returns (loss, BassKernelResults)."""
    in_maps = _make_in_maps(path_pred, path_gt, cr_pred, cr_gt)
    nc = _get_nc()
    res = run_bass_kernel_spmd(nc, in_maps, list(range(NCORES)), trace=True, **kw)
    return _combine(res.results), res



# revision 4
# speedup vs baseline: 1.2478x; 1.2478x over previous
"""Trainium2 Bass kernel for nn_ComboLoss (MTP loss + BCE loss).

Data-parallel over 8 NeuronCores: each core processes 8192 rows of the
65536-row batch and produces two partial sums [sum(ce + reg), sum(bce_raw)];
the host combines them into the final scalar loss.

v2 design (vs the fp32 baseline):
  * bf16 data plane for the big tensors (trajectories, ground truth): halves
    HBM traffic and doubles DVE throughput (2x packed mode).  All per-row
    bookkeeping (eligibility, argmin, cross-entropy, BCE) stays fp32, fed by
    small host-prepared side arrays (last waypoints, logits).
  * host-side deinterleave of (x, y) waypoint coords: per mode the row
    layout becomes [x0..x49, y0..y49], so the per-waypoint dx^2+dy^2 is a
    contiguous-halves tensor_tensor add instead of a stride-2 op.
  * per-supertile work split across engines: mode-delta subtract on
    gpsimd+vector, squares split scalar/vector, sqrt on scalar, reduces on
    vector; 4 supertiles of 2048 rows pipeline against the DMA loads.
  * best-mode trajectory fetched with one indirect DMA (8192 row-gathers of
    200 B) from the bf16 DRAM copy; smooth-L1 runs on two 32-row-group
    chunks using sum(sqrt(max(d^2,1))) - 100 == sum(relu(|d|-1)).
"""

import math
import os
import sys
from contextlib import ExitStack

import numpy as np

for _p in ("/opt/trn_rl_repo", "/root/.axon_site/_ro/trn_rl_repo"):
    if os.path.isdir(_p) and _p not in sys.path:
        sys.path.insert(0, _p)
        break

import ml_dtypes

import concourse.bass as bass
import concourse.bacc as bacc
import concourse.mybir as mybir
import concourse.tile as tile
from concourse.bass_utils import run_bass_kernel_spmd

F32 = mybir.dt.float32
BF16 = mybir.dt.bfloat16
I32 = mybir.dt.int32
ALU = mybir.AluOpType
ACTF = mybir.ActivationFunctionType
AX = mybir.AxisListType

B = 65536
NCORES = 8
BLOC = B // NCORES          # 8192 rows per core
P = 128                     # SBUF partitions
G = 16                      # row-groups per partition per supertile
ROWS_SUP = P * G            # 2048 rows per supertile
NSUP = BLOC // ROWS_SUP     # 4 supertiles
NM = 5                      # modes
T = 50                      # waypoints
T2 = 2 * T                  # 100 coords per trajectory
TF = NM * T2                # 500 trajectory coords per row (deinterleaved)
NJ = NSUP * G               # 64 row-groups per partition over the whole core
NJH = NJ // 2               # tail chunk size (row-groups)

BIG = 1.0e30
INV_COS5SQ = float(1.0 / (math.cos(math.radians(5.0)) ** 2))


def _build_bass():
    nc = bacc.Bacc("TRN2", target_bir_lowering=False, debug=False)

    pred_d = nc.dram_tensor("pred_bf", [BLOC, TF], BF16, kind="ExternalInput").ap()
    gt_d = nc.dram_tensor("gt_bf", [BLOC, T2], BF16, kind="ExternalInput").ap()
    tlx_d = nc.dram_tensor("tlx", [P, NJ * NM], F32, kind="ExternalInput").ap()
    tly_d = nc.dram_tensor("tly", [P, NJ * NM], F32, kind="ExternalInput").ap()
    lgt_d = nc.dram_tensor("lgt", [P, NJ * NM], F32, kind="ExternalInput").ap()
    glx_d = nc.dram_tensor("glx", [P, NJ], F32, kind="ExternalInput").ap()
    gly_d = nc.dram_tensor("gly", [P, NJ], F32, kind="ExternalInput").ap()
    crp_d = nc.dram_tensor("cr_pred", [P, NJ], F32, kind="ExternalInput").ap()
    crg_d = nc.dram_tensor("cr_gt", [P, NJ], F32, kind="ExternalInput").ap()
    rnd_d = nc.dram_tensor("rand_modes", [P, NJ], F32, kind="ExternalInput").ap()
    out_d = nc.dram_tensor("partials", [1, 2], F32, kind="ExternalOutput").ap()

    with tile.TileContext(nc) as tc, ExitStack() as ctx:
        cpool = ctx.enter_context(tc.tile_pool(name="const", bufs=1))
        inp = ctx.enter_context(tc.tile_pool(name="inp", bufs=2))
        wrk = ctx.enter_context(tc.tile_pool(name="wrk", bufs=2))
        sml = ctx.enter_context(tc.tile_pool(name="sml", bufs=1))
        pps = ctx.enter_context(tc.tile_pool(name="pps", bufs=1, space="PSUM"))

        # ---- constants ----
        iota_i = cpool.tile([P, NM], I32)
        nc.gpsimd.iota(iota_i[:], pattern=[[1, NM]], base=0, channel_multiplier=0)
        iota_a = cpool.tile([P, NM], F32)          # [0,1,2,3,4]
        nc.vector.tensor_copy(iota_a[:], iota_i[:])
        iota_di = cpool.tile([P, NM], I32)
        nc.gpsimd.iota(iota_di[:], pattern=[[-1, NM]], base=NM, channel_multiplier=0)
        iota_d = cpool.tile([P, NM], F32)          # [5,4,3,2,1]
        nc.vector.tensor_copy(iota_d[:], iota_di[:])
        ones = cpool.tile([P, 1], F32)
        nc.vector.memset(ones[:], 1.0)
        # element offset of each row-group's trajectory block: row*TF
        # (row = i*2048 + p*16 + g for j = i*16+g)
        rb_i = cpool.tile([P, NJ], I32)
        nc.gpsimd.iota(
            rb_i[:],
            pattern=[[ROWS_SUP, NSUP], [1, G]],
            base=0,
            channel_multiplier=G,
        )
        rb_f = cpool.tile([P, NJ], F32)
        nc.vector.tensor_copy(rb_f[:], rb_i[:])
        nc.vector.tensor_scalar(rb_f[:], rb_f[:], float(TF), None, ALU.mult)

        # ---- resident inputs ----
        rnd_sb = cpool.tile([P, NJ], F32)
        nc.sync.dma_start(rnd_sb[:], rnd_d)
        crp_sb = cpool.tile([P, NJ], F32)
        nc.sync.dma_start(crp_sb[:], crp_d)
        crg_sb = cpool.tile([P, NJ], F32)
        nc.sync.dma_start(crg_sb[:], crg_d)
        tlx_sb = cpool.tile([P, NJ * NM], F32)
        nc.sync.dma_start(tlx_sb[:], tlx_d)
        tly_sb = cpool.tile([P, NJ * NM], F32)
        nc.sync.dma_start(tly_sb[:], tly_d)
        lgt_sb = cpool.tile([P, NJ * NM], F32)
        nc.sync.dma_start(lgt_sb[:], lgt_d)
        glx_sb = cpool.tile([P, NJ], F32)
        nc.sync.dma_start(glx_sb[:], glx_d)
        gly_sb = cpool.tile([P, NJ], F32)
        nc.sync.dma_start(gly_sb[:], gly_d)
        # whole gt shard, deinterleaved [x0..x49, y0..y49] per row
        gtB = cpool.tile([P, NJ * T2], BF16)
        gt_src = gt_d.rearrange("(i p g) t -> p i g t", i=NSUP, p=P, g=G)
        nc.sync.dma_start(gtB[:], gt_src)
        gtJ = gtB[:].rearrange("p (j t) -> p j t", j=NJ)       # (P, NJ, T2)

        distB = cpool.tile([P, NJ * NM], BF16)
        distJ = distB[:].rearrange("p (j m) -> p j m", j=NJ)
        stack2 = cpool.tile([P, 2], F32)

        # ============ Phase A: per-supertile dense work ============
        with nc.allow_low_precision("bf16 tail sums; errors average out over B"):
            for i in range(NSUP):
                rsl = slice(i * ROWS_SUP, (i + 1) * ROWS_SUP)
                jsl = slice(i * G, (i + 1) * G)

                pred_t = inp.tile([P, G * TF], BF16, tag="pred")
                nc.sync.dma_start(
                    pred_t[:], pred_d[rsl, :].rearrange("(p g) f -> p (g f)", p=P)
                )
                predg = pred_t[:].rearrange("p (g f) -> p g f", g=G)
                gn = gtJ[:, jsl, :]                             # (P, G, T2)

                # deltas d[g, m, :] = traj_m - gt; modes split gpsimd/vector
                d_t = wrk.tile([P, G * NM * T2], BF16, tag="d")
                d4 = d_t[:].rearrange("p (g m t) -> p g m t", g=G, m=NM)
                for m in range(NM):
                    eng = nc.gpsimd if m < 2 else nc.vector
                    eng.tensor_tensor(
                        d4[:, :, m, :],
                        predg[:, :, m * T2:(m + 1) * T2],
                        gn,
                        ALU.subtract,
                    )

                # square in place: x-half on scalar, y-half on vector
                d5 = d_t[:].rearrange(
                    "p (g m c t) -> p g m c t", g=G, m=NM, c=2
                )
                dx = d5[:, :, :, 0, :]
                dy = d5[:, :, :, 1, :]
                nc.scalar.activation(dx, dx, ACTF.Square)
                nc.vector.tensor_tensor(dy, dy, dy, ALU.mult)

                # per-waypoint dist^2 = dx^2 + dy^2 (contiguous halves), sqrt,
                # then sum over waypoints -> per-mode distance
                e_t = wrk.tile([P, G * NM * T], BF16, tag="e")
                e3 = e_t[:].rearrange("p (gm t) -> p gm t", gm=G * NM)
                e4 = e_t[:].rearrange("p (g m t) -> p g m t", g=G, m=NM)
                nc.vector.tensor_tensor(e4, dx, dy, ALU.add)
                nc.scalar.activation(e_t[:], e_t[:], ACTF.Sqrt)
                nc.vector.tensor_reduce(
                    distB[:, i * G * NM:(i + 1) * G * NM], e3, axis=AX.X, op=ALU.add
                )

            # ============ Phase B: batched per-row small ops (fp32) ========
            tlxJ = tlx_sb[:].rearrange("p (j m) -> p j m", j=NJ)
            tlyJ = tly_sb[:].rearrange("p (j m) -> p j m", j=NJ)
            lgJ = lgt_sb[:].rearrange("p (j m) -> p j m", j=NJ)

            nt2 = sml.tile([P, NJ * NM], F32)
            nt2J = nt2[:].rearrange("p (j m) -> p j m", j=NJ)
            ty2 = sml.tile([P, NJ * NM], F32)
            nc.vector.tensor_tensor(nt2[:], tlx_sb[:], tlx_sb[:], ALU.mult)
            nc.vector.tensor_tensor(ty2[:], tly_sb[:], tly_sb[:], ALU.mult)
            nc.vector.tensor_tensor(nt2[:], nt2[:], ty2[:], ALU.add)

            nr2 = sml.tile([P, NJ], F32)
            gy2 = sml.tile([P, NJ], F32)
            nc.gpsimd.tensor_tensor(nr2[:], glx_sb[:], glx_sb[:], ALU.mult)
            nc.gpsimd.tensor_tensor(gy2[:], gly_sb[:], gly_sb[:], ALU.mult)
            nc.gpsimd.tensor_tensor(nr2[:], nr2[:], gy2[:], ALU.add)

            glx_b = glx_sb[:].unsqueeze(2).broadcast_to((P, NJ, NM))
            gly_b = gly_sb[:].unsqueeze(2).broadcast_to((P, NJ, NM))
            a1 = sml.tile([P, NJ * NM], F32)
            a1J = a1[:].rearrange("p (j m) -> p j m", j=NJ)
            nc.vector.tensor_tensor(a1J, tlxJ, glx_b, ALU.mult)
            a2 = sml.tile([P, NJ * NM], F32)
            a2J = a2[:].rearrange("p (j m) -> p j m", j=NJ)
            nc.vector.tensor_tensor(a2J, tlyJ, gly_b, ALU.mult)
            dot = sml.tile([P, NJ * NM], F32)
            nc.vector.tensor_tensor(dot[:], a1[:], a2[:], ALU.add)

            rhs = sml.tile([P, NJ * NM], F32)
            rhsJ = rhs[:].rearrange("p (j m) -> p j m", j=NJ)
            nr2_b = nr2[:].unsqueeze(2).broadcast_to((P, NJ, NM))
            nc.vector.tensor_tensor(rhsJ, nt2J, nr2_b, ALU.mult)
            lhs = sml.tile([P, NJ * NM], F32)
            nc.vector.scalar_tensor_tensor(
                lhs[:], dot[:], INV_COS5SQ, dot[:], ALU.mult, ALU.mult
            )
            e1 = sml.tile([P, NJ * NM], F32)
            nc.vector.tensor_tensor(e1[:], lhs[:], rhs[:], ALU.is_ge)
            elig = sml.tile([P, NJ * NM], F32)
            nc.vector.scalar_tensor_tensor(
                elig[:], dot[:], 0.0, e1[:], ALU.is_gt, ALU.mult
            )

            welig = sml.tile([P, NJ * NM], F32)
            nc.vector.tensor_scalar(welig[:], elig[:], -BIG, BIG, ALU.mult, ALU.add)
            distF = sml.tile([P, NJ * NM], F32)
            nc.scalar.copy(distF[:], distB[:])
            score = sml.tile([P, NJ * NM], F32)
            scoreJ = score[:].rearrange("p (j m) -> p j m", j=NJ)
            nc.vector.tensor_tensor(score[:], distF[:], welig[:], ALU.add)
            minv = sml.tile([P, NJ], F32)
            nc.vector.tensor_reduce(minv[:], scoreJ, axis=AX.X, op=ALU.min)
            eq = sml.tile([P, NJ * NM], F32)
            eqJ = eq[:].rearrange("p (j m) -> p j m", j=NJ)
            minv_b = minv[:].unsqueeze(2).broadcast_to((P, NJ, NM))
            nc.vector.tensor_tensor(eqJ, scoreJ, minv_b, ALU.is_equal)
            wq = sml.tile([P, NJ * NM], F32)
            wqJ = wq[:].rearrange("p (j m) -> p j m", j=NJ)
            iotaD_b = iota_d[:].unsqueeze(1).broadcast_to((P, NJ, NM))
            nc.vector.tensor_tensor(wqJ, eqJ, iotaD_b, ALU.mult)
            mxw = sml.tile([P, NJ], F32)
            nc.vector.tensor_reduce(mxw[:], wqJ, axis=AX.X, op=ALU.max)
            bidx = sml.tile([P, NJ], F32)
            nc.vector.tensor_scalar(
                bidx[:], mxw[:], -1.0, float(NM), ALU.mult, ALU.add
            )
            anye = sml.tile([P, NJ], I32)
            nc.vector.tensor_scalar(anye[:], minv[:], BIG, None, ALU.is_lt)
            bf = sml.tile([P, NJ], F32)
            nc.vector.tensor_copy(bf[:], rnd_sb[:])
            nc.vector.copy_predicated(bf[:], anye[:], bidx[:])

            # ===== gather best trajectory rows (bf16) via indirect DMA =====
            idxf = sml.tile([P, NJ], F32)
            nc.vector.scalar_tensor_tensor(
                idxf[:], bf[:], float(T2), rb_f[:], ALU.mult, ALU.add
            )
            idxi = sml.tile([P, NJ], I32)
            nc.vector.tensor_copy(idxi[:], idxf[:])

            db_t = cpool.tile([P, NJ * T2], BF16)
            pred_flat = pred_d.rearrange("r f -> (r f)").unsqueeze(0)
            nc.gpsimd.indirect_dma_start(
                out=db_t[:],
                out_offset=None,
                in_=pred_flat,
                in_offset=bass.IndirectOffsetOnAxis(ap=idxi[:], axis=1),
            )

            # ---- ce pieces while the gather is in flight ----
            mask = sml.tile([P, NJ * NM], F32)
            maskJ = mask[:].rearrange("p (j m) -> p j m", j=NJ)
            iotaA_b = iota_a[:].unsqueeze(1).broadcast_to((P, NJ, NM))
            bf_b = bf[:].unsqueeze(2).broadcast_to((P, NJ, NM))
            nc.vector.tensor_tensor(maskJ, iotaA_b, bf_b, ALU.is_equal)

            mxl = sml.tile([P, NJ], F32)
            nc.vector.tensor_reduce(mxl[:], lgJ, axis=AX.X, op=ALU.max)
            sh = sml.tile([P, NJ * NM], F32)
            shJ = sh[:].rearrange("p (j m) -> p j m", j=NJ)
            mxl_b = mxl[:].unsqueeze(2).broadcast_to((P, NJ, NM))
            nc.vector.tensor_tensor(shJ, lgJ, mxl_b, ALU.subtract)
            nc.scalar.activation(sh[:], sh[:], ACTF.Exp)
            se = sml.tile([P, NJ], F32)
            nc.vector.tensor_reduce(
                se[:], shJ, axis=AX.X, op=ALU.add
            )
            nc.scalar.activation(se[:], se[:], ACTF.Ln)         # lse (minus mxl)
            lbt = sml.tile([P, NJ * NM], F32)
            lbtJ = lbt[:].rearrange("p (j m) -> p j m", j=NJ)
            nc.vector.tensor_tensor(lbtJ, lgJ, maskJ, ALU.mult)
            lb = sml.tile([P, NJ], F32)
            nc.vector.tensor_reduce(lb[:], lbtJ, axis=AX.X, op=ALU.add)
            ce = sml.tile([P, NJ], F32)
            nc.vector.tensor_tensor(ce[:], mxl[:], lb[:], ALU.subtract)
            nc.vector.tensor_tensor(ce[:], ce[:], se[:], ALU.add)

            # ---- BCE on gpsimd (independent of everything above) ----
            lp = sml.tile([P, NJ], F32)
            nc.scalar.activation(lp[:], crp_sb[:], ACTF.Ln)
            nc.gpsimd.tensor_scalar(lp[:], lp[:], -100.0, None, ALU.max)
            om = sml.tile([P, NJ], F32)
            nc.gpsimd.tensor_scalar(om[:], crp_sb[:], -1.0, 1.0, ALU.mult, ALU.add)
            nc.scalar.activation(om[:], om[:], ACTF.Ln)
            nc.gpsimd.tensor_scalar(om[:], om[:], -100.0, None, ALU.max)
            u_t = sml.tile([P, NJ], F32)
            nc.gpsimd.tensor_tensor(u_t[:], lp[:], om[:], ALU.subtract)
            nc.gpsimd.tensor_tensor(u_t[:], crg_sb[:], u_t[:], ALU.mult)
            nc.gpsimd.tensor_tensor(u_t[:], u_t[:], om[:], ALU.add)
            nc.vector.tensor_reduce(
                stack2[:, 1:2], u_t[:], axis=AX.X, op=ALU.add
            )

            # ===== smooth-L1 tail on the gathered rows, 2 chunks =====
            qred = sml.tile([P, NJ], BF16)
            tredp = sml.tile([P, NJ], BF16)
            dbJ = db_t[:].rearrange("p (j t) -> p j t", j=NJ)
            for c in range(2):
                jc = slice(c * NJH, (c + 1) * NJH)
                dbc = dbJ[:, jc, :]                              # (P, NJH, T2)
                nc.vector.tensor_tensor(dbc, dbc, gtJ[:, jc, :], ALU.subtract)
                db5 = dbc.rearrange("p j (c t) -> p j c t", c=2)
                cx = db5[:, :, 0, :]
                cy = db5[:, :, 1, :]
                nc.scalar.activation(cx, cx, ACTF.Square)
                nc.vector.tensor_tensor(cy, cy, cy, ALU.mult)
                q_t = wrk.tile([P, NJH * T2], BF16, tag="q")
                nc.vector.tensor_scalar(q_t[:], dbc.rearrange("p j t -> p (j t)"),
                                        1.0, None, ALU.min)
                nc.vector.tensor_reduce(
                    qred[:, jc], q_t[:].rearrange("p (j t) -> p j t", j=NJH),
                    axis=AX.X, op=ALU.add,
                )
                nc.vector.tensor_scalar(dbc, dbc, 1.0, None, ALU.max)
                nc.scalar.activation(dbc, dbc, ACTF.Sqrt)
                nc.vector.tensor_reduce(
                    tredp[:, jc], dbc, axis=AX.X, op=ALU.add,
                )

            # reg = tredp/T2 - 1 + qred/(2*T2); total = ce + reg
            qredF = sml.tile([P, NJ], F32)
            nc.scalar.copy(qredF[:], qred[:])
            tredF = sml.tile([P, NJ], F32)
            nc.scalar.copy(tredF[:], tredp[:])
            nc.vector.scalar_tensor_tensor(
                ce[:], tredF[:], 1.0 / T2, ce[:], ALU.mult, ALU.add
            )
            nc.vector.scalar_tensor_tensor(
                ce[:], qredF[:], 0.5 / T2, ce[:], ALU.mult, ALU.add
            )
            nc.vector.tensor_scalar(ce[:], ce[:], -1.0, None, ALU.add)
            nc.vector.tensor_reduce(stack2[:, 0:1], ce[:], axis=AX.X, op=ALU.add)

        ps = pps.tile([1, 2], F32)
        nc.tensor.matmul(ps[:], ones[:], stack2[:], start=True, stop=True)
        fin = cpool.tile([1, 2], F32)
        nc.scalar.copy(fin[:], ps[:])
        nc.sync.dma_start(out_d, fin[:])

    nc.compile()
    return nc


_NC_CACHE = None


def _get_nc():
    global _NC_CACHE
    if _NC_CACHE is None:
        _NC_CACHE = _build_bass()
    return _NC_CACHE


def _rand_modes_full() -> np.ndarray:
    """The reference's fallback modes: jax.random.randint(key(42), (B,), 0, 5)."""
    import jax

    cpu = jax.devices("cpu")[0]
    with jax.default_device(cpu):
        r = jax.random.randint(jax.random.key(42), (B,), 0, NM)
        return np.asarray(jax.device_get(r)).astype(np.float32)


def _to_pj(a: np.ndarray) -> np.ndarray:
    """(BLOC, ...) row-major -> (P, NJ*...) with row = i*2048 + p*16 + g."""
    inner = a.shape[1:] if a.ndim > 1 else ()
    k = int(np.prod(inner)) if inner else 1
    return np.ascontiguousarray(
        a.reshape(NSUP, P, G, k).transpose(1, 0, 2, 3).reshape(P, NJ * k)
    )


def _make_in_maps(path_pred, path_gt, cr_pred, cr_gt):
    pp = np.asarray(path_pred, dtype=np.float32)
    pg = np.asarray(path_gt, dtype=np.float32).reshape(B, T, 2)

    traj = pp[:, :TF].reshape(B, NM, T, 2)
    # deinterleave: per mode [x0..x49, y0..y49]
    pred_bf = np.ascontiguousarray(
        traj.transpose(0, 1, 3, 2).reshape(B, TF)
    ).astype(ml_dtypes.bfloat16)
    gt_bf = np.ascontiguousarray(
        pg.transpose(0, 2, 1).reshape(B, T2)
    ).astype(ml_dtypes.bfloat16)

    tlx = np.ascontiguousarray(traj[:, :, T - 1, 0])            # (B, NM) f32
    tly = np.ascontiguousarray(traj[:, :, T - 1, 1])
    lgt = np.ascontiguousarray(pp[:, TF:TF + NM])
    glx = np.ascontiguousarray(pg[:, T - 1, 0])                 # (B,) f32
    gly = np.ascontiguousarray(pg[:, T - 1, 1])
    crp = np.asarray(cr_pred, dtype=np.float32).reshape(B)
    crg = np.asarray(cr_gt, dtype=np.float32).reshape(B)
    rnd = _rand_modes_full()

    in_maps = []
    for c in range(NCORES):
        sl = slice(c * BLOC, (c + 1) * BLOC)
        in_maps.append(
            {
                "pred_bf": np.ascontiguousarray(pred_bf[sl]),
                "gt_bf": np.ascontiguousarray(gt_bf[sl]),
                "tlx": _to_pj(tlx[sl]),
                "tly": _to_pj(tly[sl]),
                "lgt": _to_pj(lgt[sl]),
                "glx": _to_pj(glx[sl]),
                "gly": _to_pj(gly[sl]),
                "cr_pred": _to_pj(crp[sl]),
                "cr_gt": _to_pj(crg[sl]),
                "rand_modes": _to_pj(rnd[sl]),
            }
        )
    return in_maps


def _combine(results) -> np.float32:
    tot_main = 0.0
    tot_bce = 0.0
    for r in results:
        p = np.asarray(r["partials"], dtype=np.float64)
        tot_main += p[0, 0]
        tot_bce += p[0, 1]
    return np.float32(tot_main / B - tot_bce / B)


def kernel(path_pred, path_gt, cr_pred, cr_gt, log_vars=None, **_ignored):
    in_maps = _make_in_maps(path_pred, path_gt, cr_pred, cr_gt)
    nc = _get_nc()
    res = run_bass_kernel_spmd(nc, in_maps, list(range(NCORES)))
    return _combine(res.results)


def kernel_traced(path_pred, path_gt, cr_pred, cr_gt, log_vars=None, **kw):
    """Like kernel() but with NTFF profiling; returns (loss, BassKernelResults)."""
    in_maps = _make_in_maps(path_pred, path_gt, cr_pred, cr_gt)
    nc = _get_nc()
    res = run_bass_kernel_spmd(nc, in_maps, list(range(NCORES)), trace=True, **kw)
    return _combine(res.results), res
